# revision 12
# baseline (speedup 1.0000x reference)
"""Trainium2 Bass kernel for CosineAttention.

Model (fp32 reference):
  q = l2norm_head(x @ Wq.T + bq), k = l2norm_head(x @ Wk.T + bk), v = x @ Wv.T + bv
  attn = softmax(tau_h * (q . k) + mask), out = (attn @ v) @ Wo.T + bo

Sharding: B*H = 2*16 = 32 (batch, head) units over 8 cores -> each core owns
one batch (b = core//4) and 4 heads. q/k/v projections are column-sharded
(rows of W), the output projection row-sharded; partial [DM, T] outputs are
summed per batch on the host (the unshard step). tau shards with heads.

All heavy matmuls run with bf16 operands (1 cycle/row on the PE, half the
SBUF stream bandwidth of fp32r) accumulating in fp32 PSUM. All layout
transposes are done on the HOST (free): the kernel receives x^T and
pre-transposed weights, so the PE never runs a transpose:

  Q^T/K^T [hd, t] come straight out of matmuls with Wq^T/Wk^T chunks as the
  stationary operand and x^T as the moving operand; the q/k bias is fused
  into the PSUM->SBUF copy (per-partition scalar bias). Head L2 norms are
  computed by a ones-blockdiag matmul over the squared projections
  (partition-dim reduction), sqrt on Scalar, 1/x via the fast custom-DVE
  reciprocal, and broadcast back over the 64 head partitions by a tiny K=8
  matmul whose stationary also folds in tau (f32r to keep the per-row scale
  coherent error tiny). V is computed in natural [t, hd] layout using x^T
  tiles as the stationary operand, with an appended ones-column giving the
  softmax denominator for free in PSUM row 64.

Scores are computed transposed: S^T[k, q] = k-hat @ q-hat^T so softmax needs
no max subtraction (|tau*cos| <= tau) and exp(S^T) feeds the AV matmul
directly as the moving operand. Causal masks are detected on the host and
lowered to (a) skipping fully masked S^T blocks and (b) multiplying diagonal
blocks by precomputed 0/1 staircase patterns. Zero masks skip masking
entirely; arbitrary masks fall back to streaming mask^T blocks added pre-exp.
"""

import numpy as np
from contextlib import ExitStack

import ml_dtypes

import concourse.bass as bass
import concourse.mybir as mybir
import concourse.tile as tile
from concourse import bacc
from concourse.bass_utils import run_bass_kernel_spmd

B, T, DM, H = 2, 2048, 1024, 16
D = 64
NCORES = 8
HPC = 4            # heads per core
HD = HPC * D       # 256 head dims per core
NT = T // 128      # 16 k-blocks / T-tiles
NQC = T // 512     # 4 q-chunks
NKD = DM // 128    # 8 contraction chunks of DM
F32 = mybir.dt.float32
F32R = mybir.dt.float32r
BF16 = mybir.dt.bfloat16
AF = mybir.ActivationFunctionType
BF = ml_dtypes.bfloat16


def build_program(variant: str, bz: bool) -> bass.Bass:
    """variant: 'causal' | 'zeros' | 'mask'; bz: all biases are zero"""
    assert variant in ("causal", "zeros", "mask")
    nc = bacc.Bacc("TRN2", target_bir_lowering=False, debug=False,
                   num_devices=NCORES)

    xt_p = nc.declare_dram_parameter("xt", [DM, T], BF16, isOutput=False)
    wqkt_p = nc.declare_dram_parameter("wqkt", [DM, 2 * HD], BF16, isOutput=False)
    wvt_p = nc.declare_dram_parameter("wvt", [DM, HD], BF16, isOutput=False)
    wot_p = nc.declare_dram_parameter("wot", [HD, DM], BF16, isOutput=False)
    bqk_p = nc.declare_dram_parameter("bqk", [128, 4], F32, isOutput=False)
    bvr_p = nc.declare_dram_parameter("bvr", [1, HD], BF16, isOutput=False)
    bo_p = nc.declare_dram_parameter("bo", [128, NKD], F32, isOutput=False)
    tblk_p = nc.declare_dram_parameter("tblk", [8, 512], F32, isOutput=False)
    oblk_p = nc.declare_dram_parameter("oblk", [128, 4, 8], BF16, isOutput=False)
    if variant == "causal":
        pat_p = nc.declare_dram_parameter("pat", [128, 4, 512], BF16, isOutput=False)
    if variant == "mask":
        mt_p = nc.declare_dram_parameter("maskt", [T, T], F32, isOutput=False)
    yt_p = nc.declare_dram_parameter("yt", [DM, T], F32, isOutput=True)

    with tile.TileContext(nc) as tc, ExitStack() as top:
        const = top.enter_context(tc.tile_pool(name="const", bufs=1))
        wts = top.enter_context(tc.tile_pool(name="wts", bufs=1))
        acts = top.enter_context(tc.tile_pool(name="acts", bufs=1))

        # ---- weights / activations resident in SBUF ----
        wqkt = wts.tile([128, NKD, 2 * HD], BF16, tag="wqkt")
        wvt = wts.tile([128, NKD, HD], BF16, tag="wvt")
        wot = wts.tile([128, 2, DM], BF16, tag="wot")
        xts = wts.tile([128, NKD, T], BF16, tag="xts")

        qhatT = acts.tile([128, 2, T], BF16, tag="qhatT")
        khatT = acts.tile([128, 2, T], BF16, tag="khatT")
        v_ext = acts.tile([128, NT, HPC, D + 1], BF16, tag="v_ext")
        aT = acts.tile([128, 2, T], BF16, tag="aT")

        # ---- DMA: order matters (earliest consumers first) ----
        for dk in range(NKD):
            nc.sync.dma_start(wqkt[:, dk, :],
                              wqkt_p.ap()[dk * 128:(dk + 1) * 128, :])
            nc.sync.dma_start(xts[:, dk, 0:512],
                              xt_p.ap()[dk * 128:(dk + 1) * 128, 0:512])
        nc.sync.dma_start(wvt[:],
                          wvt_p.ap().rearrange("(a p) f -> p a f", p=128))

        bqk = const.tile([128, 4], F32)
        nc.sync.dma_start(bqk[:], bqk_p.ap())
        bvr = const.tile([1, HD], BF16)
        nc.sync.dma_start(bvr[:], bvr_p.ap())
        tblk_f = const.tile([8, 512], F32)
        nc.sync.dma_start(tblk_f[:], tblk_p.ap())
        oblk = const.tile([128, 4, 8], BF16)
        nc.sync.dma_start(oblk[:], oblk_p.ap())
        if variant == "causal":
            pat_sb = const.tile([128, 4, 512], BF16)
            nc.sync.dma_start(pat_sb[:], pat_p.ap())
        for tc_i in range(1, NQC):
            for dk in range(NKD):
                nc.sync.dma_start(
                    xts[:, dk, tc_i * 512:(tc_i + 1) * 512],
                    xt_p.ap()[dk * 128:(dk + 1) * 128,
                              tc_i * 512:(tc_i + 1) * 512])
        nc.sync.dma_start(wot[:],
                          wot_p.ap().rearrange("(a p) f -> p a f", p=128))
        bo_sb = const.tile([128, NKD], F32)
        nc.sync.dma_start(bo_sb[:], bo_p.ap())

        # ---- small constants ----
        ones_f = const.tile([1, 128], F32)
        nc.vector.memset(ones_f[:], 1.0)
        ones_b = const.tile([1, 128], BF16)
        nc.vector.tensor_copy(ones_b[:], ones_f[:])
        ones64_r = const.tile([1, 64], F32R)
        nc.vector.tensor_copy(ones64_r[:], ones_f[:, 0:64])
        tblk_r = const.tile([8, 512], F32R)
        nc.vector.tensor_copy(tblk_r[:], tblk_f[:])
        onesv_f = const.tile([128, NT * HPC], F32)
        nc.vector.memset(onesv_f[:], 1.0)
        # ones column of v_ext (softmax denominator trick)
        nc.vector.tensor_copy(
            v_ext[:, :, :, D:D + 1],
            onesv_f[:].rearrange("p (a b c) -> p a b c", a=NT, b=HPC))

        # ---- pools ----
        qkpool = top.enter_context(tc.tile_pool(name="qkpool", bufs=2))
        sqpool = top.enter_context(tc.tile_pool(name="sqpool", bufs=2))
        snpool = top.enter_context(tc.tile_pool(name="snpool", bufs=2))
        espool = top.enter_context(tc.tile_pool(name="espool", bufs=4))
        rpool = top.enter_context(tc.tile_pool(name="rpool", bufs=2))
        ypool = top.enter_context(tc.tile_pool(name="ypool", bufs=3))
        if variant == "mask":
            mpool = top.enter_context(tc.tile_pool(name="mpool", bufs=NT))
        # PSUM budget (8 banks): proj 2 + norm 1 + denom-bcast 1 + S/y 2 + u 2
        proj_psum = top.enter_context(
            tc.tile_pool(name="proj_psum", bufs=2, space="PSUM"))
        npsum = top.enter_context(
            tc.tile_pool(name="npsum", bufs=1, space="PSUM"))
        dpsum = top.enter_context(
            tc.tile_pool(name="dpsum", bufs=1, space="PSUM"))
        spsum = top.enter_context(
            tc.tile_pool(name="spsum", bufs=2, space="PSUM"))
        upsum = top.enter_context(
            tc.tile_pool(name="upsum", bufs=2, space="PSUM"))

        def proj_chunk(tcx):
            """project tokens [tcx*512, (tcx+1)*512) -> qhatT/khatT/v_ext"""
            ts = slice(tcx * 512, (tcx + 1) * 512)
            qb = qkpool.tile([128, 4, 512], F32, tag="qb", name=f"qb{tcx}")
            sq = sqpool.tile([128, 4, 512], BF16, tag="sq", name=f"sq{tcx}")
            for j in range(4):
                pj = proj_psum.tile([128, 512], F32, tag="proj",
                                    name=f"pj{tcx}_{j}")
                for dk in range(NKD):
                    nc.tensor.matmul(pj[:],
                                     wqkt[:, dk, j * 128:(j + 1) * 128],
                                     xts[:, dk, ts],
                                     start=(dk == 0), stop=(dk == NKD - 1))
                nc.scalar.activation(qb[:, j, :], pj[:], AF.Identity,
                                     bias=bqk[:, j:j + 1])
                nc.scalar.activation(sq[:, j, :], pj[:], AF.Square,
                                     bias=bqk[:, j:j + 1])
            ns = npsum.tile([8, 512], F32, tag="np", name=f"ns{tcx}")
            for j in range(4):
                nc.tensor.matmul(ns[:], oblk[:, j, :], sq[:, j, :],
                                 start=(j == 0), stop=(j == 3))
            # rsqrt = exp(-0.5*ln): short chain, both on Scalar
            sn = snpool.tile([8, 512], F32, tag="sn", name=f"sn{tcx}")
            nc.scalar.activation(sn[:], ns[:], AF.Ln)
            rra = snpool.tile([8, 512], F32, tag="rra", name=f"rra{tcx}")
            nc.scalar.activation(rra[:], sn[:], AF.Exp, scale=-0.5)
            rr = snpool.tile([8, 512], F32R, tag="rr", name=f"rr{tcx}")
            nc.vector.tensor_copy(rr[:], rra[:])
            # V in natural layout; x^T tiles are the stationary operand.
            # Emitted before the norm broadcasts so the PE has cover while
            # the Ln/Exp/copy chain drains.
            for tt in range(4):
                t = tcx * 4 + tt
                pv = proj_psum.tile([128, HD], F32, tag="proj",
                                    name=f"pv{t}")
                for dk in range(NKD):
                    nc.tensor.matmul(pv[:],
                                     xts[:, dk, t * 128:(t + 1) * 128],
                                     wvt[:, dk, :],
                                     start=(dk == 0), stop=(bz and dk == NKD - 1))
                if not bz:
                    nc.tensor.matmul(pv[:], ones_b[:], bvr[:],
                                     start=False, stop=True)
                nc.vector.tensor_copy(v_ext[:, t, :, 0:D],
                                      pv[:].rearrange("p (h d) -> p h d", h=HPC))
            for j in range(4):
                pool = npsum if j % 2 == 0 else dpsum
                bcp = pool.tile([128, 512], F32, tag="np" if j % 2 == 0 else "d",
                                name=f"bcp{tcx}_{j}")
                nc.tensor.matmul(bcp[:], tblk_r[:, j * 128:(j + 1) * 128],
                                 rr[:], start=True, stop=True)
                dst = qhatT if j < 2 else khatT
                nc.vector.tensor_mul(dst[:, j % 2, ts], qb[:, j, :], bcp[:])

        proj_chunk(0)

        for qc in range(NQC):
            kbs = list(range(4 * qc + 4)) if variant == "causal" else list(range(NT))
            mks = {}
            if variant == "mask":
                for kb in kbs:
                    mk = mpool.tile([128, 512], F32, tag="mk",
                                    name=f"mk{qc}_{kb}")
                    nc.sync.dma_start(
                        mk[:], mt_p.ap()[kb * 128:(kb + 1) * 128,
                                         qc * 512:(qc + 1) * 512])
                    mks[kb] = mk

            def norm_head(h, u_h):
                """finish softmax for head h: broadcast 1/denom, write aT"""
                hp, ho = h // 2, (h % 2) * 64
                bcd = dpsum.tile([64, 512], F32, tag="d", name=f"bcd{qc}_{h}")
                nc.tensor.matmul(bcd[:], ones64_r[:], rdrs[h][:],
                                 start=True, stop=True)
                bcs = rpool.tile([64, 512], F32, tag="bcs", name=f"bcs{qc}_{h}")
                nc.vector.tensor_copy(bcs[:], bcd[:])
                nc.vector.tensor_mul(
                    aT[ho:ho + 64, hp, qc * 512:(qc + 1) * 512],
                    u_h[0:D, :], bcs[:])

            def score_mm(h, kb):
                hp, ho = h // 2, (h % 2) * 64
                sp = spsum.tile([128, 512], F32, tag="s",
                                name=f"s{qc}_{kb}_{h}")
                nc.tensor.matmul(
                    sp[:],
                    khatT[ho:ho + 64, hp, kb * 128:(kb + 1) * 128],
                    qhatT[ho:ho + 64, hp, qc * 512:(qc + 1) * 512],
                    start=True, stop=True)
                return sp

            def exp_av(h, kb, sp, u_h):
                if variant == "mask":
                    nc.vector.tensor_add(sp[:], sp[:], mks[kb][:])
                es = espool.tile([128, 512], BF16, tag="es")
                nc.scalar.activation(es[:], sp[:], AF.Exp)
                if variant == "causal" and kb >= 4 * qc:
                    nc.vector.tensor_mul(es[:], es[:],
                                         pat_sb[:, kb - 4 * qc, :])
                nc.tensor.matmul(u_h[:], v_ext[:, kb, h, :], es[:],
                                 start=(kb == kbs[0]), stop=(kb == kbs[-1]))

            # software-pipelined emission: score(kb+1) is emitted BEFORE
            # exp/AV(kb) so the in-order PE queue never head-of-line blocks
            # on the exp of the current block.
            us, rdrs = [], []
            for h in range(HPC):
                u_h = upsum.tile([D + 1, 512], F32, tag="u", name=f"u{h}_{qc}")
                us.append(u_h)
                pend = None
                for kb in kbs:
                    sp = score_mm(h, kb)
                    if pend is not None:
                        exp_av(h, pend[0], pend[1], u_h)
                    pend = (kb, sp)
                exp_av(h, pend[0], pend[1], u_h)
                # denominator reciprocal right away (fast custom-DVE op)
                rdf = rpool.tile([1, 512], F32, tag="rdf", name=f"rdf{qc}_{h}")
                nc.vector.reciprocal(rdf[:], u_h[D:D + 1, :])
                rdr = rpool.tile([1, 512], F32R, tag="rdr", name=f"rdr{qc}_{h}")
                nc.vector.tensor_copy(rdr[:], rdf[:])
                rdrs.append(rdr)
                # normalize the PREVIOUS head (its reciprocal has drained)
                if h >= 1:
                    norm_head(h - 1, us[h - 1])

            # next projection chunk keeps PE busy while the last recip drains
            if qc + 1 < NQC:
                proj_chunk(qc + 1)
            norm_head(HPC - 1, us[HPC - 1])

            # output projection for this chunk
            for jt in range(NKD):
                yp = spsum.tile([128, 512], F32, tag="s", name=f"y{qc}_{jt}")
                for kc in range(2):
                    nc.tensor.matmul(yp[:], wot[:, kc, jt * 128:(jt + 1) * 128],
                                     aT[:, kc, qc * 512:(qc + 1) * 512],
                                     start=(kc == 0), stop=(kc == 1))
                ys = ypool.tile([128, 512], F32, tag="ys")
                if bz:
                    nc.vector.tensor_copy(ys[:], yp[:])
                else:
                    nc.scalar.activation(ys[:], yp[:], AF.Identity,
                                         bias=bo_sb[:, jt:jt + 1])
                nc.sync.dma_start(
                    yt_p.ap()[jt * 128:(jt + 1) * 128,
                              qc * 512:(qc + 1) * 512], ys[:])

    nc.compile()
    return nc


_PROGRAM_CACHE: dict = {}


def _get_program(variant: str, bz: bool = True) -> bass.Bass:
    key = (variant, bz)
    if key not in _PROGRAM_CACHE:
        _PROGRAM_CACHE[key] = build_program(variant, bz)
    return _PROGRAM_CACHE[key]


def _detect_variant(mask: np.ndarray) -> str:
    m = np.asarray(mask).reshape(T, T)
    if not m.any():
        return "zeros"
    tri = np.tril(np.ones((T, T), dtype=bool))
    if np.all(m[tri] == 0.0) and np.all(m[~tri] <= -1e8):
        return "causal"
    return "mask"


def _staircase_patterns() -> np.ndarray:
    kk = np.arange(128)[:, None, None]
    ai = np.arange(4)[None, :, None]
    qq = np.arange(512)[None, None, :]
    return (kk + ai * 128 <= qq).astype(BF)


def build_core_inputs(variant, x, mask, Wq, bq, Wk, bk, Wv, bv, Wo, bo, tau):
    """Host-side shard + pre-transpose + bf16 cast of all per-core inputs."""
    x = np.asarray(x, dtype=np.float32)
    Wq = np.asarray(Wq, dtype=np.float32)
    Wk = np.asarray(Wk, dtype=np.float32)
    Wv = np.asarray(Wv, dtype=np.float32)
    Wo = np.asarray(Wo, dtype=np.float32)
    bq = np.asarray(bq, dtype=np.float32)
    bk = np.asarray(bk, dtype=np.float32)
    bv = np.asarray(bv, dtype=np.float32)
    bo = np.asarray(bo, dtype=np.float32)
    tau = np.asarray(tau, dtype=np.float32).reshape(H)

    pat = _staircase_patterns() if variant == "causal" else None
    maskt = (np.ascontiguousarray(
        np.asarray(mask, dtype=np.float32).reshape(T, T).T)
        if variant == "mask" else None)

    oblk = np.zeros((128, 4, 8), dtype=BF)
    for j in range(4):
        oblk[0:64, j, 2 * j] = 1
        oblk[64:128, j, 2 * j + 1] = 1

    in_maps = []
    for c in range(NCORES):
        b = c // 4
        h0 = (c % 4) * HPC
        sl = slice(h0 * D, (h0 + HPC) * D)
        tblk = np.zeros((8, 512), dtype=np.float32)
        for j in range(4):
            v0 = tau[h0 + 2 * j] if j < 2 else 1.0
            v1 = tau[h0 + 2 * j + 1] if j < 2 else 1.0
            tblk[2 * j, j * 128:j * 128 + 64] = v0
            tblk[2 * j + 1, j * 128 + 64:(j + 1) * 128] = v1
        bqk = np.stack([bq[sl][0:128], bq[sl][128:256],
                        bk[sl][0:128], bk[sl][128:256]], axis=1)
        m = {
            "xt": np.ascontiguousarray(x[b].T).astype(BF),
            "wqkt": np.concatenate(
                [Wq[sl].T, Wk[sl].T], axis=1).astype(BF),
            "wvt": np.ascontiguousarray(Wv[sl].T).astype(BF),
            "wot": np.ascontiguousarray(Wo[:, sl].T).astype(BF),
            "bqk": np.ascontiguousarray(bqk),
            "bvr": bv[sl].reshape(1, HD).astype(BF),
            "bo": (bo.reshape(NKD, 128).T.copy() if c % 4 == 0
                   else np.zeros((128, NKD), dtype=np.float32)),
            "tblk": tblk,
            "oblk": oblk,
        }
        if variant == "causal":
            m["pat"] = pat
        if variant == "mask":
            m["maskt"] = maskt
        in_maps.append(m)
    return in_maps


def kernel(x, mask, Wq, bq, Wk, bk, Wv, bv, Wo, bo, tau):
    variant = _detect_variant(np.asarray(mask, dtype=np.float32))
    bz = not (np.asarray(bq).any() or np.asarray(bk).any()
              or np.asarray(bv).any() or np.asarray(bo).any())
    nc = _get_program(variant, bz)
    in_maps = build_core_inputs(variant, x, mask, Wq, bq, Wk, bk,
                                Wv, bv, Wo, bo, tau)
    res = run_bass_kernel_spmd(nc, in_maps, list(range(NCORES)))
    out = np.empty((B, T, DM), dtype=np.float32)
    for b in range(B):
        acc = res.results[4 * b]["yt"].copy()
        for c in range(4 * b + 1, 4 * b + 4):
            acc += res.results[c]["yt"]
        out[b] = acc.T
    return out
